# revision 36
# baseline (speedup 1.0000x reference)
"""Deformable self-attention TRN2 kernel.

Sharding: 8 cores = batch(4) x head-group(2).  Each core handles one batch
element and 4 heads (128 of 256 v/out channels), producing a partial
out-projection; the host sums the two partials per batch and adds out_b.

Per-core pipeline (Tile-scheduled):
  1. x[b] -> SBUF, cast to bf16 zero-padded [128, 2cc, 98*98]
  2. conv3x3 (48 ch: px/py/aw) + base-coord matmul + v-proj, all bf16 on PE
  3. chain (DVE/ACT): floor/frac, masks, gather indices (int16, wrapped),
     softmax(aw), bilinear corner weights w4q
  4. token tables: per head, 4 XBAR dma-transposes build a token-major
     quad table [tok, 4d x 32ch] in SBUF, DMA'd to DRAM.  Corner weights
     are XBAR-transposed to pixel-major w4T [pix%128, (d, blk, hp)].
  5. gathers: 32 non-transpose dma_gathers from the DRAM tables, spread
     over 4 SWDGE queues (desc-gen runs on 4 Q7 core pairs in parallel).
     Plain CME descriptors - no XBAR - so concurrency is safe.
  6. per (head, half): DVE weighted corner sum using 0-stride channel
     broadcast of w4T, point sum, then one batched PE transpose per
     4-block group back to channel-major `sampled`, then out-proj.

Engine-op SBUF APs must start at partition 0/32/64/96 (HW quadrant rule);
DMAs may use arbitrary partition ranges.  XBAR rules: transposes that
write interleaved slices of one tile must stay on ONE HWDGE engine
(per-engine FIFO serializes them); disjoint tiles may use both engines
concurrently, and plain CME DMAs/gathers never disturb the XBAR.
"""

import numpy as np
import ml_dtypes

H = 96
W = 96
HW = H * W          # 9216
NH = 8
NP = 4
DIM = 256
HD = 32
NCORES = 8
NHL = 4             # heads per core
PAD = 98
NPIX_PAD = PAD * PAD
CH = 2304           # chain pixel quarter
NC384 = 24
TOKR = 73
TOK = TOKR * 128    # 9344 tokens
VPADL = 97
VBUF = 9472
IDXMAX = 9312.0
HALF = HW // 2      # 4608
NBH = 36            # 128-px blocks per half

_CACHE: dict = {}


def _build_nc():
    import concourse.bacc as bacc
    import concourse.bass as bass
    import concourse.mybir as mybir
    import concourse.tile as tile

    f32 = mybir.dt.float32
    bf16 = mybir.dt.bfloat16
    i16 = mybir.dt.int16

    nc = bacc.Bacc("TRN2", target_bir_lowering=False, debug=False,
                   enable_asserts=False, num_devices=NCORES,
                   num_swdge_queues=4)

    x_d = nc.dram_tensor("x", [DIM, HW], f32, kind="ExternalInput").ap()
    convw_d = nc.dram_tensor("convw", [128, 2, 9, 48], bf16,
                             kind="ExternalInput").ap()
    cbias_d = nc.dram_tensor("cbias", [48, 1], f32, kind="ExternalInput").ap()
    basew_d = nc.dram_tensor("basew", [2, 48], bf16, kind="ExternalInput").ap()
    basein_d = nc.dram_tensor("basein", [2, HW], bf16,
                              kind="ExternalInput").ap()
    vw_d = nc.dram_tensor("vw", [128, 2, 128], bf16, kind="ExternalInput").ap()
    vb_d = nc.dram_tensor("vb", [128, 1], f32, kind="ExternalInput").ap()
    w2_d = nc.dram_tensor("w2", [128, 2, 128], bf16, kind="ExternalInput").ap()
    sumsel_d = nc.dram_tensor("sumsel", [64, 16], f32,
                              kind="ExternalInput").ap()
    bcastsel_d = nc.dram_tensor("bcastsel", [16, 64], f32,
                                kind="ExternalInput").ap()
    ident_d = nc.dram_tensor("ident", [128, 128], bf16,
                             kind="ExternalInput").ap()
    out_d = nc.dram_tensor("out", [DIM, HW], f32, kind="ExternalOutput").ap()

    with tile.TileContext(nc) as tc:
        with tc.tile_pool(name="consts", bufs=1) as cpool:
            convw = cpool.tile([128, 2, 9, 48], bf16)
            nc.sync.dma_start(out=convw, in_=convw_d)
            cbias = cpool.tile([48, 1], f32)
            nc.sync.dma_start(out=cbias, in_=cbias_d)
            basew = cpool.tile([2, 48], bf16)
            nc.sync.dma_start(out=basew, in_=basew_d)
            vw = cpool.tile([128, 2, 128], bf16)
            nc.sync.dma_start(out=vw, in_=vw_d)
            vb = cpool.tile([128, 1], f32)
            nc.sync.dma_start(out=vb, in_=vb_d)
            w2 = cpool.tile([128, 2, 128], bf16)
            nc.sync.dma_start(out=w2, in_=w2_d)
            sumsel = cpool.tile([64, 16], f32)
            nc.sync.dma_start(out=sumsel, in_=sumsel_d)
            bcastsel = cpool.tile([16, 64], f32)
            nc.sync.dma_start(out=bcastsel, in_=bcastsel_d)
            ident = cpool.tile([128, 128], bf16)
            nc.sync.dma_start(out=ident, in_=ident_d)

            with tc.tile_pool(name="persist", bufs=1) as pp, \
                 tc.tile_pool(name="dpool", bufs=1, space="DRAM") as dp:
                idx_wrap = pp.tile([128, HW], i16)
                idx16 = pp.tile([64, CH], i16)
                # pixel-major corner weights: [pix%128, (d, r18, 16q+hp)];
                # global block = 18q + r, pixel = 128*blk + part
                w4T = pp.tile([128, 4, 18, 64], bf16)
                tok_dram = [dp.tile([TOK, 128], bf16, name=f"tokd{h}")
                            for h in range(NHL)]

                with tc.tile_pool(name="vpool", bufs=1) as vp:
                    v_cm = [vp.tile([64, VBUF], bf16, name=f"v_cm{i}")
                            for i in range(2)]
                    for i in range(2):
                        nc.vector.memset(v_cm[i][:, 0:VPADL], 0.0)
                        nc.vector.memset(v_cm[i][:, VPADL + HW:VBUF], 0.0)
                    w4q = [vp.tile([64, CH], bf16, name=f"w4q{i}")
                           for i in range(4)]

                    with tc.tile_pool(name="pw", bufs=1) as pw:
                        PXY = pw.tile([128, CH], f32)
                        AWp = pw.tile([128, CH], bf16)
                        _phase1(nc, tc, bass, mybir, x_d, basein_d, convw,
                                cbias, basew, vw, vb, v_cm, PXY, AWp)
                        _phase2_chain(nc, tc, bass, mybir, PXY, AWp, sumsel,
                                      bcastsel, w4q, idx16, idx_wrap)

                    # XBAR window: transposes only.  Any plain HWDGE CME
                    # DMA concurrent with an XBAR transpose (even on the
                    # other engine) corrupts the spray - fence both sides.
                    with tc.tile_pool(name="tokp", bufs=1) as tp:
                        tok_sb = [tp.tile([128, TOKR, 128], bf16,
                                          name=f"toksb{h}")
                                  for h in range(NHL)]
                        tc.strict_bb_all_engine_barrier()
                        # the XBAR is a single shared context: ALL
                        # transposes ride one engine (FIFO-serialized);
                        # concurrent transposes on the other engine (or
                        # any HWDGE CME DMA) corrupt the spray
                        for h in range(NHL):
                            vt = v_cm[h // 2]
                            r0 = 32 * (h % 2)
                            for c, dlt in enumerate((0, 1, 96, 97)):
                                nc.sync.dma_start(
                                    out=tok_sb[h][:, :, 32 * c:32 * c + 32],
                                    in_=vt[r0:r0 + 32, dlt:dlt + TOK],
                                    transpose=True)
                        # weight slabs -> pixel-major w4T
                        # out[p, r, 16q+hp] = w4q[c][16q+hp, 128r+p]
                        for c in range(4):
                            nc.sync.dma_start(out=w4T[:, c, :, :],
                                              in_=w4q[c][:],
                                              transpose=True)
                        tc.strict_bb_all_engine_barrier()
                        # table-outs + idx DMAs ride HWDGE now that the
                        # XBAR is quiet; h0's first so its gathers can
                        # start earliest
                        for h in range(NHL):
                            eng = nc.sync if h % 2 == 0 else nc.scalar
                            # p-major: token t' = 73*p + r lands contiguous
                            # per partition (one big descriptor each)
                            eng.dma_start(
                                out=tok_dram[h][:].rearrange(
                                    "(p r) c -> p r c", r=TOKR),
                                in_=tok_sb[h])
                _phase3(nc, tc, bass, mybir, tok_dram, idx_wrap, w4T,
                        ident, w2, out_d)

    nc.compile()
    return nc


def _phase1(nc, tc, bass, mybir, x_d, basein_d, convw, cbias, basew, vw, vb,
            v_cm, PXY, AWp):
    f32 = mybir.dt.float32
    bf16 = mybir.dt.bfloat16
    Act = mybir.ActivationFunctionType

    with tc.tile_pool(name="xpad", bufs=1) as xp, \
         tc.tile_pool(name="ps1", bufs=3, space="PSUM") as ps1, \
         tc.tile_pool(name="ps2", bufs=3, space="PSUM") as ps2:
        # bf16 image, zero-padded; cast rides the SWDGE DMA (Pool is idle
        # here - the gathers come much later).  Only the 1-px border needs
        # zeroing.
        x_b = xp.tile([128, 2, NPIX_PAD], bf16)
        xb4 = x_b[:].rearrange("p cc (r c) -> p cc r c", c=PAD)
        nc.vector.memset(xb4[:, :, 0, :], 0.0)
        nc.vector.memset(xb4[:, :, 97, :], 0.0)
        nc.vector.memset(xb4[:, :, 1:97, 0:1], 0.0)
        nc.vector.memset(xb4[:, :, 1:97, 97:98], 0.0)
        for cc in range(2):
            dst = x_b[:, cc, :].rearrange("p (r c) -> p r c", c=PAD)
            xsrc = x_d[cc * 128:(cc + 1) * 128, :].rearrange(
                "p (r c) -> p r c", c=W)
            for rk in range(4):
                nc.gpsimd.dma_start(
                    out=dst[:, 1 + 24 * rk:1 + 24 * (rk + 1), 1:97],
                    in_=xsrc[:, 24 * rk:24 * (rk + 1), :])
        xv = [x_b[:, cc, :].rearrange("p (r c) -> p r c", c=PAD)
              for cc in range(2)]
        basein = xp.tile([2, HW], bf16)
        nc.sync.dma_start(out=basein, in_=basein_d)

        for n in range(NC384):
            ps = ps1.tile([48, 384], f32, tag="convps")
            first = True
            for cc in range(2):
                for t in range(9):
                    ky, kx = t // 3, t % 3
                    rhs = xv[cc][:, 4 * n + ky:4 * n + ky + 4, kx:kx + 96]
                    nc.tensor.matmul(
                        out=ps, lhsT=convw[:, cc, t, :], rhs=rhs,
                        start=first, stop=False)
                    first = False
            nc.tensor.matmul(out=ps, lhsT=basew,
                             rhs=basein[:, 384 * n:384 * (n + 1)],
                             start=False, stop=True)
            q, j6 = n // 6, n % 6
            sl = slice(384 * j6, 384 * (j6 + 1))
            nc.scalar.activation(out=PXY[32 * q:32 * q + 32, sl],
                                 in_=ps[0:32, :], func=Act.Identity,
                                 bias=cbias[0:32], scale=1.0)
            nc.scalar.activation(out=AWp[32 * q:32 * q + 16, sl],
                                 in_=ps[32:48, :], func=Act.Identity,
                                 bias=cbias[32:48], scale=1.0)

            psv = ps2.tile([128, 384], f32, tag="vps")
            for cc in range(2):
                rhsv = xv[cc][:, 4 * n + 1:4 * n + 5, 1:97]
                nc.tensor.matmul(out=psv, lhsT=vw[:, cc, :], rhs=rhsv,
                                 start=(cc == 0), stop=(cc == 1))
            nc.vector.tensor_scalar(
                out=v_cm[0][:, VPADL + 384 * n:VPADL + 384 * (n + 1)],
                in0=psv[0:64, :], scalar1=vb[0:64], scalar2=None,
                op0=mybir.AluOpType.add)
            nc.vector.tensor_scalar(
                out=v_cm[1][:, VPADL + 384 * n:VPADL + 384 * (n + 1)],
                in0=psv[64:128, :], scalar1=vb[64:128], scalar2=None,
                op0=mybir.AluOpType.add)


def _idx_dmas(nc, bass, idx16, idx_wrap):
    """Unwrap idx16 rows into the gather-ready idx_wrap layout (post-XBAR
    window; overlaps table-outs and the first gathers)."""
    engines = [nc.sync, nc.scalar]
    k = 0
    for hp in range(16):
        for q in range(4):
            row = idx16[16 * q + hp:16 * q + hp + 1, :]
            src = bass.AP(tensor=row.tensor, offset=row.offset,
                          ap=[row.ap[0], [144, 16], [1, 144]])
            engines[k % 2].dma_start(
                out=idx_wrap[0:16,
                             576 * hp + 144 * q:576 * hp + 144 * (q + 1)],
                in_=src)
            k += 1
    # tree replication of the wrapped rows: 16 -> 32 -> 64 -> 128
    # (each SWDGE queue pair reads its own 32-partition group)
    nc.sync.dma_start(out=idx_wrap[16:32, :], in_=idx_wrap[0:16, :])
    nc.scalar.dma_start(out=idx_wrap[32:64, :], in_=idx_wrap[0:32, :])
    nc.sync.dma_start(out=idx_wrap[64:128, :], in_=idx_wrap[0:64, :])


def _phase2_chain(nc, tc, bass, mybir, PXY, AWp, sumsel, bcastsel,
                  w4q, idx16, idx_wrap):
    """Weight/index chain.  Index stream first (gates the gathers), then
    softmax + corner weights into the w4q slabs (transposed to
    pixel-major in the XBAR window after this phase).

    Slab layout: x rows 16q+hp (0:64), y rows 64+16q+hp (64:128).
    """
    f32 = mybir.dt.float32
    bf16 = mybir.dt.bfloat16
    i16 = mybir.dt.int16
    i32 = mybir.dt.int32
    Alu = mybir.AluOpType
    Act = mybir.ActivationFunctionType

    with tc.tile_pool(name="chain", bufs=1) as chp, \
         tc.tile_pool(name="ps3", bufs=2, space="PSUM") as ps3, \
         tc.tile_pool(name="ps4", bufs=2, space="PSUM") as ps4:
        def t128(tag, dt=f32):
            return chp.tile([128, CH], dt, tag=tag, name=f"ch_{tag}")

        def t64(tag, dt=f32):
            return chp.tile([64, CH], dt, tag=tag, name=f"ch_{tag}")

        # top repack: PXY packed -> slab (row permutation only)
        PXYs = t128("cA")
        for q in range(4):
            nc.sync.dma_start(out=PXYs[16 * q:16 * q + 16, :],
                              in_=PXY[32 * q:32 * q + 16, :])
            nc.sync.dma_start(out=PXYs[64 + 16 * q:64 + 16 * q + 16, :],
                              in_=PXY[32 * q + 16:32 * q + 32, :])

        # aw repack early so the softmax is not queue-starved later
        aws = t64("cH", bf16)
        for q in range(4):
            nc.scalar.dma_start(out=aws[16 * q:16 * q + 16, :],
                                in_=AWp[32 * q:32 * q + 16, :])

        # floor/frac
        r32t = t128("cB", i32)
        nc.vector.tensor_copy(out=r32t, in_=PXYs)
        rf = t128("cC")
        nc.vector.tensor_copy(out=rf, in_=r32t)
        dg = t128("cD")
        nc.vector.tensor_tensor(out=dg, in0=rf, in1=PXYs, op=Alu.is_gt)
        fls = t128("cB")
        nc.vector.tensor_tensor(out=fls, in0=rf, in1=dg, op=Alu.subtract)
        frs = t128("cC")
        nc.vector.tensor_tensor(out=frs, in0=PXYs, in1=fls, op=Alu.subtract)

        # ---- index stream first: t = clip(y0*96 + x0 + 97, 0, 9312) ----
        flyc = t64("cK")
        nc.scalar.activation(out=flyc, in_=fls[64:128, :], func=Act.Copy,
                             scale=1.0)
        t1 = t64("cL")
        nc.vector.tensor_scalar(out=t1, in0=flyc,
                                scalar1=96.0, scalar2=97.0,
                                op0=Alu.mult, op1=Alu.add)
        t2 = t64("cM")
        nc.vector.tensor_tensor(out=t2, in0=t1, in1=fls[0:64, :], op=Alu.add)
        t3 = t64("cL")
        nc.vector.tensor_scalar(out=t3, in0=t2, scalar1=0.0, scalar2=IDXMAX,
                                op0=Alu.max, op1=Alu.min)
        # p-major token renumber: t' = 73*(t % 128) + t//128 (so the DRAM
        # table write is contiguous per partition).  i32 convert rounds to
        # nearest, so floor needs the is_gt correction.
        ft = t64("cM")
        nc.vector.tensor_scalar(out=ft, in0=t3, scalar1=1.0 / 128.0,
                                scalar2=None, op0=Alu.mult)
        fi = chp.tile([64, CH], i32, tag="cI")
        nc.vector.tensor_copy(out=fi, in_=ft)
        flr0 = t64("cK")
        nc.vector.tensor_copy(out=flr0, in_=fi)
        dg2 = t64("cI")
        nc.vector.tensor_tensor(out=dg2, in0=flr0, in1=ft, op=Alu.is_gt)
        flr = t64("cM")
        nc.vector.tensor_tensor(out=flr, in0=flr0, in1=dg2, op=Alu.subtract)
        ta = t64("cK")
        nc.vector.tensor_scalar(out=ta, in0=t3, scalar1=73.0, scalar2=None,
                                op0=Alu.mult)
        tb = t64("cL")
        nc.vector.tensor_scalar(out=tb, in0=flr, scalar1=9343.0, scalar2=None,
                                op0=Alu.mult)
        t3 = t64("cM")
        nc.vector.tensor_tensor(out=t3, in0=ta, in1=tb, op=Alu.subtract)

        # int16 cast with within-row wrap permute: out[144a + b] = in[a + 16b]
        t3a, idx16a = t3[:], idx16[:]
        in_ap = bass.AP(tensor=t3a.tensor, offset=t3a.offset,
                        ap=[t3a.ap[0], [1, 16], [16, 144]])
        out_ap = bass.AP(tensor=idx16a.tensor, offset=idx16a.offset,
                         ap=[idx16a.ap[0], [144, 16], [1, 144]])
        nc.scalar.activation(out=out_ap, in_=in_ap, func=Act.Copy, scale=1.0)
        _idx_dmas(nc, bass, idx16, idx_wrap)

        # ---- masks ----
        c0 = t128("cD")
        nc.vector.tensor_scalar(out=c0, in0=fls, scalar1=0.0, scalar2=95.0,
                                op0=Alu.max, op1=Alu.min)
        m0 = t128("cE")
        nc.vector.tensor_tensor(out=m0, in0=c0, in1=fls, op=Alu.is_equal)
        c1 = t128("cD")
        nc.vector.tensor_scalar(out=c1, in0=fls, scalar1=-1.0, scalar2=94.0,
                                op0=Alu.max, op1=Alu.min)
        m1 = t128("cF")
        nc.vector.tensor_tensor(out=m1, in0=c1, in1=fls, op=Alu.is_equal)

        omf = t128("cD")
        nc.vector.tensor_scalar(out=omf, in0=frs, scalar1=-1.0, scalar2=1.0,
                                op0=Alu.mult, op1=Alu.add)
        f0 = omf  # in place: omf * m0
        nc.vector.tensor_tensor(out=f0, in0=omf, in1=m0, op=Alu.mult)
        f1 = frs  # in place: frs * m1
        nc.vector.tensor_tensor(out=f1, in0=frs, in1=m1, op=Alu.mult)

        # softmax over points, folded into y-factors
        exps = t64("cI")
        nc.scalar.activation(out=exps, in_=aws, func=Act.Exp, scale=1.0)
        awn = t64("cH", bf16)
        for j in range(6):
            sl = slice(384 * j, 384 * (j + 1))
            pss = ps3.tile([16, 384], f32, tag="ssum")
            nc.tensor.matmul(out=pss, lhsT=sumsel, rhs=exps[:, sl],
                             start=True, stop=True)
            rsum = chp.tile([16, 384], f32, tag="cI2")
            nc.vector.reciprocal_approx_fast(out=rsum, in_=pss)
            psb = ps4.tile([64, 384], f32, tag="sbc")
            nc.tensor.matmul(out=psb, lhsT=bcastsel, rhs=rsum,
                             start=True, stop=True)
            nc.vector.tensor_tensor(out=awn[:, sl], in0=exps[:, sl],
                                    in1=psb, op=Alu.mult)

        # y factors (stage y-halves down to 0:64 for TT partition match)
        fy0c = t64("cK")
        nc.scalar.activation(out=fy0c, in_=f0[64:128, :], func=Act.Copy,
                             scale=1.0)
        f0y = t64("cL")
        nc.vector.tensor_tensor(out=f0y, in0=fy0c, in1=awn, op=Alu.mult)
        fy1c = t64("cK")
        nc.scalar.activation(out=fy1c, in_=f1[64:128, :], func=Act.Copy,
                             scale=1.0)
        f1y = t64("cM")
        nc.vector.tensor_tensor(out=f1y, in0=fy1c, in1=awn, op=Alu.mult)

        nc.vector.tensor_tensor(out=w4q[0], in0=f0[0:64, :], in1=f0y,
                                op=Alu.mult)
        nc.vector.tensor_tensor(out=w4q[1], in0=f1[0:64, :], in1=f0y,
                                op=Alu.mult)
        nc.vector.tensor_tensor(out=w4q[2], in0=f0[0:64, :], in1=f1y,
                                op=Alu.mult)
        nc.vector.tensor_tensor(out=w4q[3], in0=f1[0:64, :], in1=f1y,
                                op=Alu.mult)


def _phase3(nc, tc, bass, mybir, tok_dram, idx_wrap, w4T, ident,
            w2, out_d):
    """Non-transpose gathers (4 SWDGE queues) + DVE weighted corner/point
    reduction in token-major layout + batched PE transposes back to
    channel-major + out-projection."""
    f32 = mybir.dt.float32
    bf16 = mybir.dt.bfloat16
    Alu = mybir.AluOpType
    Act = mybir.ActivationFunctionType

    with tc.tile_pool(name="smp", bufs=1) as smp, \
         tc.tile_pool(name="gpool", bufs=2) as gp, \
         tc.tile_pool(name="spool", bufs=2) as sp, \
         tc.tile_pool(name="opool", bufs=2) as op, \
         tc.tile_pool(name="ptr", bufs=2, space="PSUM") as ptr, \
         tc.tile_pool(name="pso", bufs=2, space="PSUM") as pso:

        sampled = [smp.tile([128, CH], bf16, name=f"smp{s_}")
                   for s_ in range(4)]
        w4Ta = w4T[:]
        NBQ = 18          # 128-px blocks per quarter

        for seg in range(4):
            for h in range(NHL):
                gt = []
                for p in range(NP):
                    hp = h * 4 + p
                    g_t = gp.tile([128, NBQ, 128], bf16, tag=f"g{p}",
                                  name=f"gt{hp}_{seg}")
                    gt.append(g_t)
                    nc.gpsimd.dma_gather(
                        g_t[:], tok_dram[h][:],
                        idx_wrap[:, 576 * hp + 144 * seg:
                                 576 * hp + 144 * (seg + 1)],
                        CH, CH, 128,
                        transpose=False,
                        single_packet=False,
                        queue_num=p)

                # weighted corner sum per point (0-stride ch broadcast),
                # then point sum -> scast bf16 [128, NBQ, 32].
                # w4T free layout (d, r, 16q+hp): strides 1152, 64, 1;
                # quarter == q, so one mult per point.
                sps = []
                for p in range(NP):
                    hp = h * 4 + p
                    gv = gt[p][:].rearrange("p b (d c) -> p b d c", c=32)
                    win = bass.AP(
                        tensor=w4Ta.tensor,
                        offset=(w4Ta.offset + 16 * seg + hp),
                        ap=[w4Ta.ap[0], [64, NBQ],
                            [18 * 64, 4], [0, 32]])
                    nc.vector.tensor_tensor(out=gv, in0=gv, in1=win,
                                            op=Alu.mult)
                    a = sp.tile([128, NBQ, 32], bf16, tag="sa")
                    nc.vector.tensor_tensor(out=a, in0=gv[:, :, 0, :],
                                            in1=gv[:, :, 1, :], op=Alu.add)
                    b = sp.tile([128, NBQ, 32], bf16, tag="sb")
                    nc.vector.tensor_tensor(out=b, in0=gv[:, :, 2, :],
                                            in1=gv[:, :, 3, :], op=Alu.add)
                    s_p = sp.tile([128, NBQ, 32], bf16, tag=f"sp{p}")
                    nc.vector.tensor_tensor(out=s_p, in0=a, in1=b,
                                            op=Alu.add)
                    sps.append(s_p)
                a01 = sp.tile([128, NBQ, 32], f32, tag="pa")
                nc.vector.tensor_tensor(out=a01, in0=sps[0], in1=sps[1],
                                        op=Alu.add)
                a23 = sp.tile([128, NBQ, 32], f32, tag="pb")
                nc.vector.tensor_tensor(out=a23, in0=sps[2], in1=sps[3],
                                        op=Alu.add)
                scast = sp.tile([128, NBQ, 32], bf16, tag="sc")
                nc.vector.tensor_tensor(out=scast, in0=a01, in1=a23,
                                        op=Alu.add)

                # batched PE transposes: [128 pix, 3blk x 32ch] -> psum
                # [3blk x 32ch, 128 pix]; drain per 32-row group into
                # sampled[32h.., cols]
                for B in range(NBQ // 3):
                    pt = ptr.tile([96, 128], bf16, tag="pt")
                    lhsT = scast[:, 3 * B:3 * B + 3, :].rearrange(
                        "p b c -> p (b c)")
                    nc.tensor.matmul(out=pt, lhsT=lhsT, rhs=ident[:],
                                     is_transpose=True, start=True, stop=True)
                    for b in range(3):
                        col0 = 128 * (3 * B + b)
                        nc.scalar.activation(
                            out=sampled[seg][32 * h:32 * h + 32,
                                             col0:col0 + 128],
                            in_=pt[32 * b:32 * b + 32, :],
                            func=Act.Copy, scale=1.0)

            # out-projection for this quarter (all 4 heads done)
            for n in range(6 * seg, 6 * (seg + 1)):
                sl = slice(384 * n, 384 * (n + 1))
                sll = slice(384 * (n - 6 * seg), 384 * (n - 6 * seg + 1))
                for oh in range(2):
                    ob = pso.tile([128, 384], f32, tag="ob")
                    nc.tensor.matmul(out=ob, lhsT=w2[:, oh, :],
                                     rhs=sampled[seg][:, sll],
                                     start=True, stop=True)
                    osb = op.tile([128, 384], f32, tag="osb")
                    if (n + oh) % 2 == 0:
                        nc.vector.tensor_copy(out=osb, in_=ob)
                    else:
                        nc.scalar.activation(out=osb, in_=ob,
                                             func=Act.Copy, scale=1.0)
                    (nc.sync if (n + oh) % 2 else nc.scalar).dma_start(
                        out=out_d[oh * 128:(oh + 1) * 128, sl],
                        in_=osb)


def _host_inputs(inputs):
    x = np.asarray(inputs["x"], dtype=np.float32)
    kv_w = np.asarray(inputs["kv_w"], dtype=np.float32)
    kv_b = np.asarray(inputs["kv_b"], dtype=np.float32)
    off_w = np.asarray(inputs["off_w"], dtype=np.float32)
    off_b = np.asarray(inputs["off_b"], dtype=np.float32)
    aw_w = np.asarray(inputs["aw_w"], dtype=np.float32)
    aw_b = np.asarray(inputs["aw_b"], dtype=np.float32)
    out_w = np.asarray(inputs["out_w"], dtype=np.float32)

    sx = (W - 1.0) / W
    sy = (H - 1.0) / H

    sumsel = np.zeros((64, 16), np.float32)
    bcastsel = np.zeros((16, 64), np.float32)
    for q in range(4):
        for hh in range(4):
            for p in range(4):
                sumsel[16 * q + 4 * hh + p, 4 * q + hh] = 1.0
                bcastsel[4 * q + hh, 16 * q + 4 * hh + p] = 1.0

    basein = np.zeros((2, HW), np.float32)
    basein[0] = np.arange(HW) % W
    basein[1] = np.arange(HW) // W
    basew = np.zeros((2, 48), np.float32)
    basew[0, 0:16] = 1.0
    basew[1, 16:32] = 1.0

    bf = ml_dtypes.bfloat16
    in_maps = []
    for core in range(NCORES):
        b, hg = core // 2, core % 2
        heads = list(range(4 * hg, 4 * hg + 4))

        convw = np.zeros((128, 2, 9, 48), np.float32)
        cbias = np.zeros((48, 1), np.float32)
        for j, gh in enumerate(heads):
            for p in range(NP):
                hp = j * 4 + p
                wx = off_w[gh * 8 + p * 2 + 0] * sx
                wy = off_w[gh * 8 + p * 2 + 1] * sy
                wa = aw_w[gh * 4 + p]
                for t in range(9):
                    ky, kx = t // 3, t % 3
                    for cc in range(2):
                        csl = slice(cc * 128, (cc + 1) * 128)
                        convw[:, cc, t, hp] = wx[csl, ky, kx]
                        convw[:, cc, t, 16 + hp] = wy[csl, ky, kx]
                        convw[:, cc, t, 32 + hp] = wa[csl, ky, kx]
                cbias[hp, 0] = off_b[gh * 8 + p * 2 + 0] * sx
                cbias[16 + hp, 0] = off_b[gh * 8 + p * 2 + 1] * sy
                cbias[32 + hp, 0] = aw_b[gh * 4 + p]

        vw = np.zeros((128, 2, 128), np.float32)
        vrows = kv_w[DIM + hg * 128:DIM + (hg + 1) * 128, :]
        for cc in range(2):
            vw[:, cc, :] = vrows[:, cc * 128:(cc + 1) * 128].T
        vb = kv_b[DIM + hg * 128:DIM + (hg + 1) * 128].reshape(128, 1)

        w2 = np.zeros((128, 2, 128), np.float32)
        for halfi in range(2):
            w2[:, halfi, :] = out_w[halfi * 128:(halfi + 1) * 128,
                                    hg * 128:(hg + 1) * 128].T

        in_maps.append({
            "x": np.ascontiguousarray(x[b]),
            "convw": convw.astype(bf),
            "cbias": cbias,
            "basew": basew.astype(bf),
            "basein": basein.astype(bf),
            "vw": vw.astype(bf),
            "vb": np.ascontiguousarray(vb),
            "w2": w2.astype(bf),
            "sumsel": sumsel,
            "bcastsel": bcastsel,
            "ident": np.eye(128, dtype=np.float32).astype(bf),
        })
    return in_maps


def kernel(**inputs):
    from concourse import bass_utils

    if "nc" not in _CACHE:
        _CACHE["nc"] = _build_nc()
    nc = _CACHE["nc"]

    in_maps = _host_inputs(inputs)
    res = bass_utils.run_bass_kernel_spmd(nc, in_maps,
                                          core_ids=list(range(NCORES)))
    out_b = np.asarray(inputs["out_b"], dtype=np.float32)
    out = np.zeros((4, DIM, HW), np.float32)
    for b in range(4):
        out[b] = (res.results[2 * b]["out"] + res.results[2 * b + 1]["out"]
                  + out_b[:, None])
    return out


# revision 37
# speedup vs baseline: 1.0063x; 1.0063x over previous
"""Deformable self-attention TRN2 kernel.

Sharding: 8 cores = batch(4) x head-group(2).  Each core handles one batch
element and 4 heads (128 of 256 v/out channels), producing a partial
out-projection; the host sums the two partials per batch and adds out_b.

Per-core pipeline (Tile-scheduled):
  1. x[b] -> SBUF, cast to bf16 zero-padded [128, 2cc, 98*98]
  2. conv3x3 (48 ch: px/py/aw) + base-coord matmul + v-proj, all bf16 on PE
  3. chain (DVE/ACT): floor/frac, masks, gather indices (int16, wrapped),
     softmax(aw), bilinear corner weights w4q
  4. token tables: per head, 4 XBAR dma-transposes build a token-major
     quad table [tok, 4d x 32ch] in SBUF, DMA'd to DRAM.  Corner weights
     are XBAR-transposed to pixel-major w4T [pix%128, (d, blk, hp)].
  5. gathers: 32 non-transpose dma_gathers from the DRAM tables, spread
     over 4 SWDGE queues (desc-gen runs on 4 Q7 core pairs in parallel).
     Plain CME descriptors - no XBAR - so concurrency is safe.
  6. per (quarter, head): DVE weighted corner sum using 0-stride channel
     broadcast of w4T, point sum, then one batched PE transpose per
     3-block group back to channel-major per-quarter `sampled`, then the
     quarter's out-projection.

Engine-op SBUF APs must start at partition 0/32/64/96 (HW quadrant rule);
DMAs may use arbitrary partition ranges.  XBAR rules: transposes that
write interleaved slices of one tile must stay on ONE HWDGE engine
(per-engine FIFO serializes them); disjoint tiles may use both engines
concurrently, and plain CME DMAs/gathers never disturb the XBAR.
"""

import numpy as np
import ml_dtypes

H = 96
W = 96
HW = H * W          # 9216
NH = 8
NP = 4
DIM = 256
HD = 32
NCORES = 8
NHL = 4             # heads per core
PAD = 98
NPIX_PAD = PAD * PAD
CH = 2304           # chain pixel quarter
NC384 = 24
TOKR = 73
TOK = TOKR * 128    # 9344 tokens
VPADL = 97
VBUF = 9472
IDXMAX = 9312.0
HALF = HW // 2      # 4608
NBH = 36            # 128-px blocks per half

_CACHE: dict = {}


def _build_nc():
    import concourse.bacc as bacc
    import concourse.bass as bass
    import concourse.mybir as mybir
    import concourse.tile as tile

    f32 = mybir.dt.float32
    bf16 = mybir.dt.bfloat16
    i16 = mybir.dt.int16

    nc = bacc.Bacc("TRN2", target_bir_lowering=False, debug=False,
                   enable_asserts=False, num_devices=NCORES,
                   num_swdge_queues=4)

    x_d = nc.dram_tensor("x", [DIM, HW], f32, kind="ExternalInput").ap()
    convw_d = nc.dram_tensor("convw", [128, 2, 9, 48], bf16,
                             kind="ExternalInput").ap()
    cbias_d = nc.dram_tensor("cbias", [48, 1], f32, kind="ExternalInput").ap()
    basew_d = nc.dram_tensor("basew", [2, 48], bf16, kind="ExternalInput").ap()
    basein_d = nc.dram_tensor("basein", [2, HW], bf16,
                              kind="ExternalInput").ap()
    vw_d = nc.dram_tensor("vw", [128, 2, 128], bf16, kind="ExternalInput").ap()
    vb_d = nc.dram_tensor("vb", [128, 1], f32, kind="ExternalInput").ap()
    w2_d = nc.dram_tensor("w2", [128, 2, 128], bf16, kind="ExternalInput").ap()
    sumsel_d = nc.dram_tensor("sumsel", [64, 16], f32,
                              kind="ExternalInput").ap()
    bcastsel_d = nc.dram_tensor("bcastsel", [16, 64], f32,
                                kind="ExternalInput").ap()
    ident_d = nc.dram_tensor("ident", [128, 128], bf16,
                             kind="ExternalInput").ap()
    out_d = nc.dram_tensor("out", [DIM, HW], f32, kind="ExternalOutput").ap()

    with tile.TileContext(nc) as tc:
        with tc.tile_pool(name="consts", bufs=1) as cpool:
            convw = cpool.tile([128, 2, 9, 48], bf16)
            nc.sync.dma_start(out=convw, in_=convw_d)
            cbias = cpool.tile([48, 1], f32)
            nc.sync.dma_start(out=cbias, in_=cbias_d)
            basew = cpool.tile([2, 48], bf16)
            nc.sync.dma_start(out=basew, in_=basew_d)
            vw = cpool.tile([128, 2, 128], bf16)
            nc.sync.dma_start(out=vw, in_=vw_d)
            vb = cpool.tile([128, 1], f32)
            nc.sync.dma_start(out=vb, in_=vb_d)
            w2 = cpool.tile([128, 2, 128], bf16)
            nc.sync.dma_start(out=w2, in_=w2_d)
            sumsel = cpool.tile([64, 16], f32)
            nc.sync.dma_start(out=sumsel, in_=sumsel_d)
            bcastsel = cpool.tile([16, 64], f32)
            nc.sync.dma_start(out=bcastsel, in_=bcastsel_d)
            ident = cpool.tile([128, 128], bf16)
            nc.sync.dma_start(out=ident, in_=ident_d)

            with tc.tile_pool(name="persist", bufs=1) as pp, \
                 tc.tile_pool(name="dpool", bufs=1, space="DRAM") as dp:
                idx_wrap = pp.tile([128, HW], i16)
                idx16 = pp.tile([64, CH], i16)
                # pixel-major corner weights: [pix%128, (d, r18, 16q+hp)];
                # global block = 18q + r, pixel = 128*blk + part
                w4T = pp.tile([128, 4, 18, 64], bf16)
                tok_dram = [dp.tile([TOK, 128], bf16, name=f"tokd{h}")
                            for h in range(NHL)]

                with tc.tile_pool(name="vpool", bufs=1) as vp:
                    v_cm = [vp.tile([64, VBUF], bf16, name=f"v_cm{i}")
                            for i in range(2)]
                    for i in range(2):
                        nc.vector.memset(v_cm[i][:, 0:VPADL], 0.0)
                        nc.vector.memset(v_cm[i][:, VPADL + HW:VBUF], 0.0)
                    w4q = [vp.tile([64, CH], bf16, name=f"w4q{i}")
                           for i in range(4)]

                    with tc.tile_pool(name="pw", bufs=1) as pw:
                        PXY = pw.tile([128, CH], f32)
                        AWp = pw.tile([128, CH], bf16)
                        _phase1(nc, tc, bass, mybir, x_d, basein_d, convw,
                                cbias, basew, vw, vb, v_cm, PXY, AWp)
                        _phase2_chain(nc, tc, bass, mybir, PXY, AWp, sumsel,
                                      bcastsel, w4q, idx16, idx_wrap)

                    # XBAR window: transposes only.  Any plain HWDGE CME
                    # DMA concurrent with an XBAR transpose (even on the
                    # other engine) corrupts the spray - fence both sides.
                    with tc.tile_pool(name="tokp", bufs=1) as tp:
                        tok_sb = [tp.tile([128, TOKR, 128], bf16,
                                          name=f"toksb{h}")
                                  for h in range(NHL)]
                        tc.strict_bb_all_engine_barrier()
                        # the XBAR is a single shared context: ALL
                        # transposes ride one engine (FIFO-serialized);
                        # concurrent transposes on the other engine (or
                        # any HWDGE CME DMA) corrupt the spray
                        for h in range(NHL):
                            vt = v_cm[h // 2]
                            r0 = 32 * (h % 2)
                            for c, dlt in enumerate((0, 1, 96, 97)):
                                nc.sync.dma_start(
                                    out=tok_sb[h][:, :, 32 * c:32 * c + 32],
                                    in_=vt[r0:r0 + 32, dlt:dlt + TOK],
                                    transpose=True)
                        # weight slabs -> pixel-major w4T
                        # out[p, r, 16q+hp] = w4q[c][16q+hp, 128r+p]
                        for c in range(4):
                            nc.sync.dma_start(out=w4T[:, c, :, :],
                                              in_=w4q[c][:],
                                              transpose=True)
                        tc.strict_bb_all_engine_barrier()
                        # table-outs + idx DMAs ride HWDGE now that the
                        # XBAR is quiet; h0's first so its gathers can
                        # start earliest
                        for h in range(NHL):
                            eng = nc.sync if h % 2 == 0 else nc.scalar
                            # p-major: token t' = 73*p + r lands contiguous
                            # per partition (one big descriptor each)
                            eng.dma_start(
                                out=tok_dram[h][:].rearrange(
                                    "(p r) c -> p r c", r=TOKR),
                                in_=tok_sb[h])
                _phase3(nc, tc, bass, mybir, tok_dram, idx_wrap, w4T,
                        ident, w2, out_d)

    nc.compile()
    return nc


def _phase1(nc, tc, bass, mybir, x_d, basein_d, convw, cbias, basew, vw, vb,
            v_cm, PXY, AWp):
    f32 = mybir.dt.float32
    bf16 = mybir.dt.bfloat16
    Act = mybir.ActivationFunctionType

    with tc.tile_pool(name="xpad", bufs=1) as xp, \
         tc.tile_pool(name="ps1", bufs=3, space="PSUM") as ps1, \
         tc.tile_pool(name="ps2", bufs=3, space="PSUM") as ps2:
        # bf16 image, zero-padded; cast rides the SWDGE DMA (Pool is idle
        # here - the gathers come much later).  Only the 1-px border needs
        # zeroing.
        x_b = xp.tile([128, 2, NPIX_PAD], bf16)
        xb4 = x_b[:].rearrange("p cc (r c) -> p cc r c", c=PAD)
        nc.vector.memset(xb4[:, :, 0, :], 0.0)
        nc.vector.memset(xb4[:, :, 97, :], 0.0)
        nc.vector.memset(xb4[:, :, 1:97, 0:1], 0.0)
        nc.vector.memset(xb4[:, :, 1:97, 97:98], 0.0)
        for cc in range(2):
            dst = x_b[:, cc, :].rearrange("p (r c) -> p r c", c=PAD)
            xsrc = x_d[cc * 128:(cc + 1) * 128, :].rearrange(
                "p (r c) -> p r c", c=W)
            for rk in range(4):
                nc.gpsimd.dma_start(
                    out=dst[:, 1 + 24 * rk:1 + 24 * (rk + 1), 1:97],
                    in_=xsrc[:, 24 * rk:24 * (rk + 1), :])
        xv = [x_b[:, cc, :].rearrange("p (r c) -> p r c", c=PAD)
              for cc in range(2)]
        basein = xp.tile([2, HW], bf16)
        nc.sync.dma_start(out=basein, in_=basein_d)

        for n in range(NC384):
            ps = ps1.tile([48, 384], f32, tag="convps")
            first = True
            for cc in range(2):
                for t in range(9):
                    ky, kx = t // 3, t % 3
                    rhs = xv[cc][:, 4 * n + ky:4 * n + ky + 4, kx:kx + 96]
                    nc.tensor.matmul(
                        out=ps, lhsT=convw[:, cc, t, :], rhs=rhs,
                        start=first, stop=False)
                    first = False
            nc.tensor.matmul(out=ps, lhsT=basew,
                             rhs=basein[:, 384 * n:384 * (n + 1)],
                             start=False, stop=True)
            q, j6 = n // 6, n % 6
            sl = slice(384 * j6, 384 * (j6 + 1))
            nc.scalar.activation(out=PXY[32 * q:32 * q + 32, sl],
                                 in_=ps[0:32, :], func=Act.Identity,
                                 bias=cbias[0:32], scale=1.0)
            nc.scalar.activation(out=AWp[32 * q:32 * q + 16, sl],
                                 in_=ps[32:48, :], func=Act.Identity,
                                 bias=cbias[32:48], scale=1.0)

            psv = ps2.tile([128, 384], f32, tag="vps")
            for cc in range(2):
                rhsv = xv[cc][:, 4 * n + 1:4 * n + 5, 1:97]
                nc.tensor.matmul(out=psv, lhsT=vw[:, cc, :], rhs=rhsv,
                                 start=(cc == 0), stop=(cc == 1))
            nc.vector.tensor_scalar(
                out=v_cm[0][:, VPADL + 384 * n:VPADL + 384 * (n + 1)],
                in0=psv[0:64, :], scalar1=vb[0:64], scalar2=None,
                op0=mybir.AluOpType.add)
            nc.vector.tensor_scalar(
                out=v_cm[1][:, VPADL + 384 * n:VPADL + 384 * (n + 1)],
                in0=psv[64:128, :], scalar1=vb[64:128], scalar2=None,
                op0=mybir.AluOpType.add)


def _idx_dmas(nc, bass, idx16, idx_wrap):
    """Unwrap idx16 rows into the gather-ready idx_wrap layout (post-XBAR
    window; overlaps table-outs and the first gathers)."""
    engines = [nc.sync, nc.scalar]
    k = 0
    for hp in range(16):
        for q in range(4):
            row = idx16[16 * q + hp:16 * q + hp + 1, :]
            src = bass.AP(tensor=row.tensor, offset=row.offset,
                          ap=[row.ap[0], [144, 16], [1, 144]])
            engines[k % 2].dma_start(
                out=idx_wrap[0:16,
                             576 * hp + 144 * q:576 * hp + 144 * (q + 1)],
                in_=src)
            k += 1
    # tree replication of the wrapped rows: 16 -> 32 -> 64 -> 128
    # (each SWDGE queue pair reads its own 32-partition group)
    nc.sync.dma_start(out=idx_wrap[16:32, :], in_=idx_wrap[0:16, :])
    nc.scalar.dma_start(out=idx_wrap[32:64, :], in_=idx_wrap[0:32, :])
    nc.sync.dma_start(out=idx_wrap[64:128, :], in_=idx_wrap[0:64, :])


def _phase2_chain(nc, tc, bass, mybir, PXY, AWp, sumsel, bcastsel,
                  w4q, idx16, idx_wrap):
    """Weight/index chain.  Index stream first (gates the gathers), then
    softmax + corner weights into the w4q slabs (transposed to
    pixel-major in the XBAR window after this phase).

    Slab layout: x rows 16q+hp (0:64), y rows 64+16q+hp (64:128).
    """
    f32 = mybir.dt.float32
    bf16 = mybir.dt.bfloat16
    i16 = mybir.dt.int16
    i32 = mybir.dt.int32
    Alu = mybir.AluOpType
    Act = mybir.ActivationFunctionType

    with tc.tile_pool(name="chain", bufs=1) as chp, \
         tc.tile_pool(name="ps3", bufs=2, space="PSUM") as ps3, \
         tc.tile_pool(name="ps4", bufs=2, space="PSUM") as ps4:
        def t128(tag, dt=f32):
            return chp.tile([128, CH], dt, tag=tag, name=f"ch_{tag}")

        def t64(tag, dt=f32):
            return chp.tile([64, CH], dt, tag=tag, name=f"ch_{tag}")

        # top repack: PXY packed -> slab (row permutation only)
        PXYs = t128("cA")
        for q in range(4):
            nc.sync.dma_start(out=PXYs[16 * q:16 * q + 16, :],
                              in_=PXY[32 * q:32 * q + 16, :])
            nc.sync.dma_start(out=PXYs[64 + 16 * q:64 + 16 * q + 16, :],
                              in_=PXY[32 * q + 16:32 * q + 32, :])

        # aw repack early so the softmax is not queue-starved later
        aws = t64("cH", bf16)
        for q in range(4):
            nc.scalar.dma_start(out=aws[16 * q:16 * q + 16, :],
                                in_=AWp[32 * q:32 * q + 16, :])

        # floor/frac
        r32t = t128("cB", i32)
        nc.vector.tensor_copy(out=r32t, in_=PXYs)
        rf = t128("cC")
        nc.vector.tensor_copy(out=rf, in_=r32t)
        dg = t128("cD")
        nc.vector.tensor_tensor(out=dg, in0=rf, in1=PXYs, op=Alu.is_gt)
        fls = t128("cB")
        nc.vector.tensor_tensor(out=fls, in0=rf, in1=dg, op=Alu.subtract)
        frs = t128("cC")
        nc.vector.tensor_tensor(out=frs, in0=PXYs, in1=fls, op=Alu.subtract)

        # ---- index stream first: t = clip(y0*96 + x0 + 97, 0, 9312) ----
        flyc = t64("cK")
        nc.scalar.activation(out=flyc, in_=fls[64:128, :], func=Act.Copy,
                             scale=1.0)
        t1 = t64("cL")
        nc.vector.tensor_scalar(out=t1, in0=flyc,
                                scalar1=96.0, scalar2=97.0,
                                op0=Alu.mult, op1=Alu.add)
        t2 = t64("cM")
        nc.vector.tensor_tensor(out=t2, in0=t1, in1=fls[0:64, :], op=Alu.add)
        t3 = t64("cL")
        nc.vector.tensor_scalar(out=t3, in0=t2, scalar1=0.0, scalar2=IDXMAX,
                                op0=Alu.max, op1=Alu.min)
        # p-major token renumber: t' = 73*(t % 128) + t//128 (so the DRAM
        # table write is contiguous per partition).  i32 convert rounds to
        # nearest, so floor needs the is_gt correction.
        ft = t64("cM")
        nc.vector.tensor_scalar(out=ft, in0=t3, scalar1=1.0 / 128.0,
                                scalar2=None, op0=Alu.mult)
        fi = chp.tile([64, CH], i32, tag="cI")
        nc.vector.tensor_copy(out=fi, in_=ft)
        flr0 = t64("cK")
        nc.vector.tensor_copy(out=flr0, in_=fi)
        dg2 = t64("cI")
        nc.vector.tensor_tensor(out=dg2, in0=flr0, in1=ft, op=Alu.is_gt)
        flr = t64("cM")
        nc.vector.tensor_tensor(out=flr, in0=flr0, in1=dg2, op=Alu.subtract)
        ta = t64("cK")
        nc.vector.tensor_scalar(out=ta, in0=t3, scalar1=73.0, scalar2=None,
                                op0=Alu.mult)
        tb = t64("cL")
        nc.vector.tensor_scalar(out=tb, in0=flr, scalar1=9343.0, scalar2=None,
                                op0=Alu.mult)
        t3 = t64("cM")
        nc.vector.tensor_tensor(out=t3, in0=ta, in1=tb, op=Alu.subtract)

        # int16 cast with within-row wrap permute: out[144a + b] = in[a + 16b]
        t3a, idx16a = t3[:], idx16[:]
        in_ap = bass.AP(tensor=t3a.tensor, offset=t3a.offset,
                        ap=[t3a.ap[0], [1, 16], [16, 144]])
        out_ap = bass.AP(tensor=idx16a.tensor, offset=idx16a.offset,
                         ap=[idx16a.ap[0], [144, 16], [1, 144]])
        nc.scalar.activation(out=out_ap, in_=in_ap, func=Act.Copy, scale=1.0)
        _idx_dmas(nc, bass, idx16, idx_wrap)

        # ---- masks ----
        c0 = t128("cD")
        nc.vector.tensor_scalar(out=c0, in0=fls, scalar1=0.0, scalar2=95.0,
                                op0=Alu.max, op1=Alu.min)
        m0 = t128("cE")
        nc.vector.tensor_tensor(out=m0, in0=c0, in1=fls, op=Alu.is_equal)
        c1 = t128("cD")
        nc.vector.tensor_scalar(out=c1, in0=fls, scalar1=-1.0, scalar2=94.0,
                                op0=Alu.max, op1=Alu.min)
        m1 = t128("cF")
        nc.vector.tensor_tensor(out=m1, in0=c1, in1=fls, op=Alu.is_equal)

        omf = t128("cD")
        nc.vector.tensor_scalar(out=omf, in0=frs, scalar1=-1.0, scalar2=1.0,
                                op0=Alu.mult, op1=Alu.add)
        f0 = omf  # in place: omf * m0
        nc.vector.tensor_tensor(out=f0, in0=omf, in1=m0, op=Alu.mult)
        f1 = frs  # in place: frs * m1
        nc.vector.tensor_tensor(out=f1, in0=frs, in1=m1, op=Alu.mult)

        # softmax over points, folded into y-factors
        exps = t64("cI")
        nc.scalar.activation(out=exps, in_=aws, func=Act.Exp, scale=1.0)
        awn = t64("cH", bf16)
        for j in range(6):
            sl = slice(384 * j, 384 * (j + 1))
            pss = ps3.tile([16, 384], f32, tag="ssum")
            nc.tensor.matmul(out=pss, lhsT=sumsel, rhs=exps[:, sl],
                             start=True, stop=True)
            rsum = chp.tile([16, 384], f32, tag="cI2")
            nc.vector.reciprocal_approx_fast(out=rsum, in_=pss)
            psb = ps4.tile([64, 384], f32, tag="sbc")
            nc.tensor.matmul(out=psb, lhsT=bcastsel, rhs=rsum,
                             start=True, stop=True)
            nc.vector.tensor_tensor(out=awn[:, sl], in0=exps[:, sl],
                                    in1=psb, op=Alu.mult)

        # y factors (stage y-halves down to 0:64 for TT partition match)
        fy0c = t64("cK")
        nc.scalar.activation(out=fy0c, in_=f0[64:128, :], func=Act.Copy,
                             scale=1.0)
        f0y = t64("cL")
        nc.vector.tensor_tensor(out=f0y, in0=fy0c, in1=awn, op=Alu.mult)
        fy1c = t64("cK")
        nc.scalar.activation(out=fy1c, in_=f1[64:128, :], func=Act.Copy,
                             scale=1.0)
        f1y = t64("cM")
        nc.vector.tensor_tensor(out=f1y, in0=fy1c, in1=awn, op=Alu.mult)

        nc.vector.tensor_tensor(out=w4q[0], in0=f0[0:64, :], in1=f0y,
                                op=Alu.mult)
        nc.vector.tensor_tensor(out=w4q[1], in0=f1[0:64, :], in1=f0y,
                                op=Alu.mult)
        nc.vector.tensor_tensor(out=w4q[2], in0=f0[0:64, :], in1=f1y,
                                op=Alu.mult)
        nc.vector.tensor_tensor(out=w4q[3], in0=f1[0:64, :], in1=f1y,
                                op=Alu.mult)


def _phase3(nc, tc, bass, mybir, tok_dram, idx_wrap, w4T, ident,
            w2, out_d):
    """Non-transpose gathers (4 SWDGE queues) + DVE weighted corner/point
    reduction in token-major layout + batched PE transposes back to
    channel-major + out-projection."""
    f32 = mybir.dt.float32
    bf16 = mybir.dt.bfloat16
    Alu = mybir.AluOpType
    Act = mybir.ActivationFunctionType

    with tc.tile_pool(name="smp", bufs=1) as smp, \
         tc.tile_pool(name="gpool", bufs=2) as gp, \
         tc.tile_pool(name="spool", bufs=2) as sp, \
         tc.tile_pool(name="opool", bufs=2) as op, \
         tc.tile_pool(name="ptr", bufs=2, space="PSUM") as ptr, \
         tc.tile_pool(name="pso", bufs=2, space="PSUM") as pso:

        sampled = [smp.tile([128, CH], bf16, name=f"smp{s_}")
                   for s_ in range(4)]
        w4Ta = w4T[:]
        NBQ = 18          # 128-px blocks per quarter

        for seg in range(4):
            for h in range(NHL):
                gt = []
                for p in range(NP):
                    hp = h * 4 + p
                    g_t = gp.tile([128, NBQ, 128], bf16, tag=f"g{p}",
                                  name=f"gt{hp}_{seg}")
                    gt.append(g_t)
                    nc.gpsimd.dma_gather(
                        g_t[:], tok_dram[h][:],
                        idx_wrap[:, 576 * hp + 144 * seg:
                                 576 * hp + 144 * (seg + 1)],
                        CH, CH, 128,
                        transpose=False,
                        single_packet=False,
                        queue_num=p)

                # weighted corner sum per point (0-stride ch broadcast),
                # then point sum -> scast bf16 [128, NBQ, 32].
                # w4T free layout (d, r, 16q+hp): strides 1152, 64, 1;
                # quarter == q, so one mult per point.
                sps = []
                for p in range(NP):
                    hp = h * 4 + p
                    gv = gt[p][:].rearrange("p b (d c) -> p b d c", c=32)
                    win = bass.AP(
                        tensor=w4Ta.tensor,
                        offset=(w4Ta.offset + 16 * seg + hp),
                        ap=[w4Ta.ap[0], [64, NBQ],
                            [18 * 64, 4], [0, 32]])
                    nc.vector.tensor_tensor(out=gv, in0=gv, in1=win,
                                            op=Alu.mult)
                    a = sp.tile([128, NBQ, 32], bf16, tag="sa")
                    nc.vector.tensor_tensor(out=a, in0=gv[:, :, 0, :],
                                            in1=gv[:, :, 1, :], op=Alu.add)
                    b = sp.tile([128, NBQ, 32], bf16, tag="sb")
                    nc.vector.tensor_tensor(out=b, in0=gv[:, :, 2, :],
                                            in1=gv[:, :, 3, :], op=Alu.add)
                    s_p = sp.tile([128, NBQ, 32], bf16, tag=f"sp{p}")
                    nc.vector.tensor_tensor(out=s_p, in0=a, in1=b,
                                            op=Alu.add)
                    sps.append(s_p)
                a01 = sp.tile([128, NBQ, 32], f32, tag="pa")
                nc.vector.tensor_tensor(out=a01, in0=sps[0], in1=sps[1],
                                        op=Alu.add)
                a23 = sp.tile([128, NBQ, 32], f32, tag="pb")
                nc.vector.tensor_tensor(out=a23, in0=sps[2], in1=sps[3],
                                        op=Alu.add)
                scast = sp.tile([128, NBQ, 32], bf16, tag="sc")
                nc.vector.tensor_tensor(out=scast, in0=a01, in1=a23,
                                        op=Alu.add)

                # batched PE transposes: [128 pix, 3blk x 32ch] -> psum
                # [3blk x 32ch, 128 pix]; drain per 32-row group into
                # sampled[32h.., cols]
                for B in range(NBQ // 3):
                    pt = ptr.tile([96, 128], bf16, tag="pt")
                    lhsT = scast[:, 3 * B:3 * B + 3, :].rearrange(
                        "p b c -> p (b c)")
                    nc.tensor.matmul(out=pt, lhsT=lhsT, rhs=ident[:],
                                     is_transpose=True, start=True, stop=True)
                    for b in range(3):
                        col0 = 128 * (3 * B + b)
                        nc.scalar.activation(
                            out=sampled[seg][32 * h:32 * h + 32,
                                             col0:col0 + 128],
                            in_=pt[32 * b:32 * b + 32, :],
                            func=Act.Copy, scale=1.0)

            # out-projection for this quarter (all 4 heads done)
            for n in range(6 * seg, 6 * (seg + 1)):
                sl = slice(384 * n, 384 * (n + 1))
                sll = slice(384 * (n - 6 * seg), 384 * (n - 6 * seg + 1))
                for oh in range(2):
                    ob = pso.tile([128, 384], f32, tag="ob")
                    nc.tensor.matmul(out=ob, lhsT=w2[:, oh, :],
                                     rhs=sampled[seg][:, sll],
                                     start=True, stop=True)
                    osb = op.tile([128, 384], f32, tag="osb")
                    if (n + oh) % 2 == 0:
                        nc.vector.tensor_copy(out=osb, in_=ob)
                    else:
                        nc.scalar.activation(out=osb, in_=ob,
                                             func=Act.Copy, scale=1.0)
                    (nc.sync if (n + oh) % 2 else nc.scalar).dma_start(
                        out=out_d[oh * 128:(oh + 1) * 128, sl],
                        in_=osb)


def _host_inputs(inputs):
    x = np.asarray(inputs["x"], dtype=np.float32)
    kv_w = np.asarray(inputs["kv_w"], dtype=np.float32)
    kv_b = np.asarray(inputs["kv_b"], dtype=np.float32)
    off_w = np.asarray(inputs["off_w"], dtype=np.float32)
    off_b = np.asarray(inputs["off_b"], dtype=np.float32)
    aw_w = np.asarray(inputs["aw_w"], dtype=np.float32)
    aw_b = np.asarray(inputs["aw_b"], dtype=np.float32)
    out_w = np.asarray(inputs["out_w"], dtype=np.float32)

    sx = (W - 1.0) / W
    sy = (H - 1.0) / H

    sumsel = np.zeros((64, 16), np.float32)
    bcastsel = np.zeros((16, 64), np.float32)
    for q in range(4):
        for hh in range(4):
            for p in range(4):
                sumsel[16 * q + 4 * hh + p, 4 * q + hh] = 1.0
                bcastsel[4 * q + hh, 16 * q + 4 * hh + p] = 1.0

    basein = np.zeros((2, HW), np.float32)
    basein[0] = np.arange(HW) % W
    basein[1] = np.arange(HW) // W
    basew = np.zeros((2, 48), np.float32)
    basew[0, 0:16] = 1.0
    basew[1, 16:32] = 1.0

    bf = ml_dtypes.bfloat16
    in_maps = []
    for core in range(NCORES):
        b, hg = core // 2, core % 2
        heads = list(range(4 * hg, 4 * hg + 4))

        convw = np.zeros((128, 2, 9, 48), np.float32)
        cbias = np.zeros((48, 1), np.float32)
        for j, gh in enumerate(heads):
            for p in range(NP):
                hp = j * 4 + p
                wx = off_w[gh * 8 + p * 2 + 0] * sx
                wy = off_w[gh * 8 + p * 2 + 1] * sy
                wa = aw_w[gh * 4 + p]
                for t in range(9):
                    ky, kx = t // 3, t % 3
                    for cc in range(2):
                        csl = slice(cc * 128, (cc + 1) * 128)
                        convw[:, cc, t, hp] = wx[csl, ky, kx]
                        convw[:, cc, t, 16 + hp] = wy[csl, ky, kx]
                        convw[:, cc, t, 32 + hp] = wa[csl, ky, kx]
                cbias[hp, 0] = off_b[gh * 8 + p * 2 + 0] * sx
                cbias[16 + hp, 0] = off_b[gh * 8 + p * 2 + 1] * sy
                cbias[32 + hp, 0] = aw_b[gh * 4 + p]

        vw = np.zeros((128, 2, 128), np.float32)
        vrows = kv_w[DIM + hg * 128:DIM + (hg + 1) * 128, :]
        for cc in range(2):
            vw[:, cc, :] = vrows[:, cc * 128:(cc + 1) * 128].T
        vb = kv_b[DIM + hg * 128:DIM + (hg + 1) * 128].reshape(128, 1)

        w2 = np.zeros((128, 2, 128), np.float32)
        for halfi in range(2):
            w2[:, halfi, :] = out_w[halfi * 128:(halfi + 1) * 128,
                                    hg * 128:(hg + 1) * 128].T

        in_maps.append({
            "x": np.ascontiguousarray(x[b]),
            "convw": convw.astype(bf),
            "cbias": cbias,
            "basew": basew.astype(bf),
            "basein": basein.astype(bf),
            "vw": vw.astype(bf),
            "vb": np.ascontiguousarray(vb),
            "w2": w2.astype(bf),
            "sumsel": sumsel,
            "bcastsel": bcastsel,
            "ident": np.eye(128, dtype=np.float32).astype(bf),
        })
    return in_maps


def kernel(**inputs):
    from concourse import bass_utils

    if "nc" not in _CACHE:
        _CACHE["nc"] = _build_nc()
    nc = _CACHE["nc"]

    in_maps = _host_inputs(inputs)
    res = bass_utils.run_bass_kernel_spmd(nc, in_maps,
                                          core_ids=list(range(NCORES)))
    out_b = np.asarray(inputs["out_b"], dtype=np.float32)
    out = np.zeros((4, DIM, HW), np.float32)
    for b in range(4):
        out[b] = (res.results[2 * b]["out"] + res.results[2 * b + 1]["out"]
                  + out_b[:, None])
    return out
